# revision 2
# baseline (speedup 1.0000x reference)
"""Trainium2 Bass kernel for nn_Attention_47029891891630.

Single-head attention: qh = q@Wq.T+bq, kh = k@Wk.T+bk, vh = v@Wv.T+bv,
scores = qh@kh.T/sqrt(768), probs = softmax(scores), out = probs@vh.
Returns (out, scores) at full shape [4,2048,768] / [4,2048,2048].

Sharding: 8 cores = (batch b, q-half h); each core computes kh/vh for its
batch (redundantly within the pair) and attention for its 1024 q rows.

Per-core dataflow (all matmuls in float32r = full-rate reduced-precision):
  Phase A: DMA q/k/v natural -> PE-transpose 128x128 tiles -> project
           -> resident qhT [e,1024], khT [e,2048] (f32r), vh [k,e] (f32r).
           1/sqrt(D) is folded into Wq/bq on the host.
  Phase B: per 128-row q-tile: scores(psum) = qhT.khT -> ACT copy->HBM,
           ACT Exp(accum_out=rowsum) -> PE-transpose probs -> out(psum) =
           probsT.vh -> DVE mul by 1/rowsum -> HBM.
"""

import os
import sys
import types
from contextlib import ExitStack

import numpy as np

import concourse.bass as bass
import concourse.mybir as mybir
import concourse.tile as tile
from concourse.bass_utils import run_bass_kernel_spmd
from concourse.masks import make_identity
from concourse.vector_clock import ScopedClock

F32 = mybir.dt.float32
F32R = mybir.dt.float32r
AF = mybir.ActivationFunctionType

B, S, D = 4, 2048, 768
SH = S // 2          # q rows per core
DC = D // 128        # 6 feature chunks
NCORES = 8
SCALE = float(np.sqrt(D))

# ---------------------------------------------------------------------------
# Patch 1: this container's walrus rejects >1 sync-wait per instruction
# ("Too many sync wait commands"). Hoist excess waits onto same-engine NOP
# carriers immediately before the instruction (identical stall semantics).
# ---------------------------------------------------------------------------
_MAX_WAITS = 1


def _split_waits(tc, inst):
    si = inst.sync_info
    if si is None or si.on_wait is None or len(si.on_wait) <= _MAX_WAITS:
        return
    waits = list(si.on_wait)
    keep = waits[-_MAX_WAITS:]
    hoist = waits[:-_MAX_WAITS]
    inst.sync_info = mybir.SyncInfo(on_wait=keep, on_update=list(si.on_update or []))
    eng = tc.nc.engines[inst.engine]
    for w in hoist:
        nop = eng.nop(nofuse=True)
        nop.ins.sync_info = mybir.SyncInfo(on_wait=[w], on_update=[])


_orig_commit = tile.TileContext._commit_instruction


def _commit_instruction_split(self, inst, lazy_reg_writes: bool = True):
    si = inst.sync_info
    if (
        si is not None
        and si.on_wait is not None
        and len(si.on_wait) > _MAX_WAITS
        and inst.engine != mybir.EngineType.Unassigned
    ):
        _split_waits(self, inst)
    return _orig_commit(self, inst, lazy_reg_writes)


tile.TileContext._commit_instruction = _commit_instruction_split


def _drain_and_barrier_split(self, tick_clock, wait_clock):
    drain_inst = self.nc.sync.drain()
    wait_clock.add_sem_waits(
        drain_inst.ins, ScopedClock({None: tick_clock.global_clock})
    )
    si = drain_inst.ins.sync_info
    if si is not None and si.on_wait is not None and len(si.on_wait) > _MAX_WAITS:
        waits = list(si.on_wait)
        drain_inst.ins.sync_info = mybir.SyncInfo(
            on_wait=[waits[0]], on_update=list(si.on_update or [])
        )
        for w in waits[1:]:
            extra = self.nc.sync.drain()
            extra.ins.sync_info = mybir.SyncInfo(on_wait=[w], on_update=[])

    self.nc.all_engine_barrier()
    assert self.sems is not None
    popped = self.nc._tile_sem_poison_stack.pop()
    assert popped is self._sem_poison
    self.nc.clear_and_free_semaphores(list(self.sems.allocated().values()))
    self.nc.all_engine_barrier()


tile.TileContext._drain_and_barrier = _drain_and_barrier_split

# ---------------------------------------------------------------------------
# Patch 2: provide antenv.axon_hooks (absent in this image) so that
# run_bass_kernel_spmd(trace=True) can NTFF-profile via libaxon_pjrt.so.
# ---------------------------------------------------------------------------
if "antenv.axon_hooks" not in sys.modules:
    _hookmod = types.ModuleType("antenv.axon_hooks")
    _hookmod._hook = None

    def _set_hook(h, _m=_hookmod):
        _m._hook = h

    def _get_hook(_m=_hookmod):
        return _m._hook

    _hookmod.set_axon_ntff_profile_hook = _set_hook
    _hookmod.get_axon_ntff_profile_hook = _get_hook
    sys.modules["antenv.axon_hooks"] = _hookmod
    try:
        from trn_agent_boot.trn_boot import _ntff_profile_via_ctypes

        _set_hook(_ntff_profile_via_ctypes("/opt/axon/libaxon_pjrt.so"))
    except Exception:
        pass


# ---------------------------------------------------------------------------
# Kernel build
# ---------------------------------------------------------------------------
def _build():
    nc = bass.Bass("TRN2", target_bir_lowering=False, debug=False,
                   num_devices=NCORES)

    q_d = nc.dram_tensor("q", [SH, D], F32, kind="ExternalInput")
    k_d = nc.dram_tensor("k", [S, D], F32, kind="ExternalInput")
    v_d = nc.dram_tensor("v", [S, D], F32, kind="ExternalInput")
    wq_d = nc.dram_tensor("wq", [D, D], F32R, kind="ExternalInput")
    wk_d = nc.dram_tensor("wk", [D, D], F32R, kind="ExternalInput")
    wv_d = nc.dram_tensor("wv", [D, D], F32R, kind="ExternalInput")
    bq_d = nc.dram_tensor("bq", [128, DC], F32, kind="ExternalInput")
    bk_d = nc.dram_tensor("bk", [128, DC], F32, kind="ExternalInput")
    bv_d = nc.dram_tensor("bv", [1, D], F32, kind="ExternalInput")
    scores_d = nc.dram_tensor("scores", [SH, S], F32, kind="ExternalOutput")
    out_d = nc.dram_tensor("out", [SH, D], F32, kind="ExternalOutput")

    with tile.TileContext(nc) as tc, ExitStack() as ctx:
        consts = ctx.enter_context(tc.tile_pool(name="consts", bufs=1))
        ident = consts.tile([128, 128], F32)
        make_identity(nc, ident)
        bq_sb = consts.tile([128, DC], F32, tag="bq")
        nc.sync.dma_start(out=bq_sb, in_=bq_d[:, :])
        bk_sb = consts.tile([128, DC], F32, tag="bk")
        nc.sync.dma_start(out=bk_sb, in_=bk_d[:, :])
        bv_rep = consts.tile([128, D], F32, tag="bv")
        nc.gpsimd.dma_start(
            out=bv_rep,
            in_=bass.AP(tensor=bv_d, offset=0, ap=[[0, 128], [1, D]]),
        )

        acts = ctx.enter_context(tc.tile_pool(name="acts", bufs=1))
        qhT = acts.tile([128, DC, SH], F32R, tag="qhT")
        khT = acts.tile([128, DC, S], F32R, tag="khT")
        vh = acts.tile([128, S // 128, D], F32R, tag="vh")

        # ---------------- Phase A1: q and k projections -------------------
        with tc.tile_pool(name="wqk", bufs=1) as wqk, \
             tc.tile_pool(name="work1", bufs=2) as work, \
             tc.tile_pool(name="psA1", bufs=2, space="PSUM") as psA:
            wq_sb = wqk.tile([128, DC, D], F32R, tag="wq")
            nc.sync.dma_start(out=wq_sb,
                              in_=wq_d[:, :].rearrange("(dc p) e -> p dc e", p=128))
            wk_sb = wqk.tile([128, DC, D], F32R, tag="wk")
            nc.sync.dma_start(out=wk_sb,
                              in_=wk_d[:, :].rearrange("(dc p) e -> p dc e", p=128))

            for src, dst, w_sb, bias_sb, nch in (
                (q_d, qhT, wq_sb, bq_sb, SH // 256),
                (k_d, khT, wk_sb, bk_sb, S // 256),
            ):
                for ch in range(nch):
                    nat = work.tile([128, 2, D], F32, tag="nat")
                    nc.sync.dma_start(
                        out=nat,
                        in_=src[ch * 256:(ch + 1) * 256, :]
                        .rearrange("(c p) e -> p c e", p=128),
                    )
                    xT = work.tile([128, DC, 256], F32R, tag="xT")
                    for dc in range(DC):
                        pst = psA.tile([128, 256], F32, tag="pst")
                        for c in range(2):
                            nc.tensor.transpose(
                                pst[:, c * 128:(c + 1) * 128],
                                nat[:, c, dc * 128:(dc + 1) * 128],
                                ident,
                            )
                        nc.vector.tensor_copy(xT[:, dc, :], pst.bitcast(F32R))
                    for ec in range(DC):
                        psp = psA.tile([128, 256], F32, tag="psp")
                        for dc in range(DC):
                            nc.tensor.matmul(
                                psp,
                                w_sb[:, dc, ec * 128:(ec + 1) * 128],
                                xT[:, dc, :],
                                start=(dc == 0),
                                stop=(dc == DC - 1),
                            )
                        nc.vector.tensor_scalar_add(
                            dst[:, ec, ch * 256:(ch + 1) * 256],
                            psp.bitcast(F32R),
                            bias_sb[:, ec:ec + 1],
                        )

        # ---------------- Phase A2: v projection (vh natural) -------------
        with tc.tile_pool(name="wvp", bufs=1) as wvp, \
             tc.tile_pool(name="work2", bufs=2) as work, \
             tc.tile_pool(name="psA2", bufs=2, space="PSUM") as psA:
            wv_sb = wvp.tile([128, DC, D], F32R, tag="wv")
            nc.sync.dma_start(out=wv_sb,
                              in_=wv_d[:, :].rearrange("(dc p) e -> p dc e", p=128))
            for kc in range(S // 128):
                natv = work.tile([128, D], F32, tag="natv")
                nc.sync.dma_start(out=natv, in_=v_d[kc * 128:(kc + 1) * 128, :])
                vT = work.tile([128, DC, 128], F32R, tag="vT")
                for g in range(2):
                    pst = psA.tile([128, 384], F32, tag="pst")
                    for j in range(3):
                        dc = g * 3 + j
                        nc.tensor.transpose(
                            pst[:, j * 128:(j + 1) * 128],
                            natv[:, dc * 128:(dc + 1) * 128],
                            ident,
                        )
                    nc.vector.tensor_copy(
                        vT[:, g * 3:(g + 1) * 3, :],
                        pst.rearrange("p (c x) -> p c x", c=3).bitcast(F32R),
                    )
                for ev in range(2):
                    psv = psA.tile([128, 384], F32, tag="psv")
                    for dc in range(DC):
                        nc.tensor.matmul(
                            psv,
                            vT[:, dc, :],
                            wv_sb[:, dc, ev * 384:(ev + 1) * 384],
                            start=(dc == 0),
                            stop=(dc == DC - 1),
                        )
                    nc.vector.tensor_add(
                        vh[:, kc, ev * 384:(ev + 1) * 384],
                        psv.bitcast(F32R),
                        bv_rep[:, ev * 384:(ev + 1) * 384].bitcast(F32R),
                    )

        # ---------------- Phase B: attention per q-tile --------------------
        with tc.tile_pool(name="att", bufs=2) as aw, \
             tc.tile_pool(name="psS", bufs=1, space="PSUM") as psS, \
             tc.tile_pool(name="psT2", bufs=2, space="PSUM") as psT2, \
             tc.tile_pool(name="psO", bufs=1, space="PSUM") as psO:
            for qt in range(SH // 128):
                pss = psS.tile([128, S], F32, tag="pss")
                for kg in range(S // 512):
                    for ec in range(DC):
                        nc.tensor.matmul(
                            pss[:, kg * 512:(kg + 1) * 512],
                            qhT[:, ec, qt * 128:(qt + 1) * 128],
                            khT[:, ec, kg * 512:(kg + 1) * 512],
                            start=(ec == 0),
                            stop=(ec == DC - 1),
                        )
                scs = aw.tile([128, S], F32, tag="scs")
                nc.scalar.copy(scs, pss)
                nc.sync.dma_start(out=scores_d[qt * 128:(qt + 1) * 128, :], in_=scs)

                probs = aw.tile([128, S], F32, tag="probs")
                den = aw.tile([128, 1], F32, tag="den")
                nc.scalar.activation(probs, scs, AF.Exp, accum_out=den)
                rden = aw.tile([128, 1], F32, tag="rden")
                nc.vector.reciprocal(rden, den)

                probsT = aw.tile([128, S // 128, 128], F32R, tag="probsT")
                for g in range(4):
                    pst = psT2.tile([128, 512], F32, tag="pst2")
                    for j in range(4):
                        t = g * 4 + j
                        nc.tensor.transpose(
                            pst[:, j * 128:(j + 1) * 128],
                            probs[:, t * 128:(t + 1) * 128],
                            ident,
                        )
                    nc.vector.tensor_copy(
                        probsT[:, g * 4:(g + 1) * 4, :],
                        pst.rearrange("p (c x) -> p c x", c=4).bitcast(F32R),
                    )

                pso = psO.tile([128, D], F32, tag="pso")
                for e0, ew in ((0, 512), (512, 256)):
                    for t in range(S // 128):
                        nc.tensor.matmul(
                            pso[:, e0:e0 + ew],
                            probsT[:, t, :],
                            vh[:, t, e0:e0 + ew],
                            start=(t == 0),
                            stop=(t == S // 128 - 1),
                        )
                outsb = aw.tile([128, D], F32, tag="outsb")
                nc.vector.tensor_scalar_mul(outsb, pso, rden[:, 0:1])
                nc.sync.dma_start(out=out_d[qt * 128:(qt + 1) * 128, :], in_=outsb)

    return nc


_NC = None


def _get_nc():
    global _NC
    if _NC is None:
        _NC = _build()
    return _NC


def kernel(q, k, v, Wq, bq, Wk, bk, Wv, bv):
    q = np.ascontiguousarray(np.asarray(q, dtype=np.float32))
    k = np.ascontiguousarray(np.asarray(k, dtype=np.float32))
    v = np.ascontiguousarray(np.asarray(v, dtype=np.float32))
    Wq = np.asarray(Wq, dtype=np.float32)
    Wk = np.asarray(Wk, dtype=np.float32)
    Wv = np.asarray(Wv, dtype=np.float32)
    bq = np.asarray(bq, dtype=np.float32)
    bk = np.asarray(bk, dtype=np.float32)
    bv = np.asarray(bv, dtype=np.float32)

    wq_p = np.ascontiguousarray((Wq / SCALE).T)          # [d, e]
    wk_p = np.ascontiguousarray(Wk.T)
    wv_p = np.ascontiguousarray(Wv.T)
    bq_p = np.ascontiguousarray((bq / SCALE).reshape(DC, 128).T)   # [128, DC]
    bk_p = np.ascontiguousarray(bk.reshape(DC, 128).T)
    bv_p = np.ascontiguousarray(bv.reshape(1, D))

    in_maps = []
    for c in range(NCORES):
        b, h = divmod(c, 2)
        in_maps.append({
            "q": np.ascontiguousarray(q[b, h * SH:(h + 1) * SH, :]),
            "k": k[b],
            "v": v[b],
            "wq": wq_p, "wk": wk_p, "wv": wv_p,
            "bq": bq_p, "bk": bk_p, "bv": bv_p,
        })

    nc = _get_nc()
    trace = bool(int(os.environ.get("ATTN_TRACE", "0")))
    res = run_bass_kernel_spmd(nc, in_maps, list(range(NCORES)), trace=trace)
    if trace:
        print("HW exec time: %s ns" % res.exec_time_ns)
        kernel.last_results = res

    outputs = np.empty((B, S, D), dtype=np.float32)
    scores = np.empty((B, S, S), dtype=np.float32)
    for c in range(NCORES):
        b, h = divmod(c, 2)
        outputs[b, h * SH:(h + 1) * SH, :] = res.results[c]["out"]
        scores[b, h * SH:(h + 1) * SH, :] = res.results[c]["scores"]
    return (outputs, scores)


# revision 6
# speedup vs baseline: 1.1192x; 1.1192x over previous
"""Trainium2 Bass kernel for nn_Attention_47029891891630.

Single-head attention: qh = q@Wq.T+bq, kh = k@Wk.T+bk, vh = v@Wv.T+bv,
scores = qh@kh.T/sqrt(768), probs = softmax(scores), out = probs@vh.
Returns (out, scores) at full shape [4,2048,768] / [4,2048,2048].

Sharding: 8 cores = (batch b, q-half h); each core computes kh/vh for its
batch (redundantly within the pair) and attention for its 1024 q rows.

Per-core dataflow (all matmuls in float32r = full-rate reduced-precision):
  Phase A: DMA q/k/v natural -> PE-transpose 128x128 tiles -> project
           -> resident qhT [e,1024], khT [e,2048] (f32r), vh [k,e] (f32r).
           1/sqrt(D) is folded into Wq/bq on the host.
  Phase B: per 128-row q-tile: scores(psum) = qhT.khT -> ACT copy->HBM,
           ACT Exp(accum_out=rowsum) -> PE-transpose probs -> out(psum) =
           probsT.vh -> DVE mul by 1/rowsum -> HBM.
"""

import os
import sys
import types
from contextlib import ExitStack

import numpy as np

import concourse.bass as bass
import concourse.mybir as mybir
import concourse.tile as tile
from concourse.bass_utils import run_bass_kernel_spmd
from concourse.vector_clock import ScopedClock

F32 = mybir.dt.float32
F32R = mybir.dt.float32r
AF = mybir.ActivationFunctionType

B, S, D = 4, 2048, 768
SH = S // 2          # q rows per core
DC = D // 128        # 6 feature chunks
NCORES = 8
SCALE = float(np.sqrt(D))

# ---------------------------------------------------------------------------
# Patch 1: this container's walrus rejects >1 sync-wait per instruction
# ("Too many sync wait commands"). Hoist excess waits onto same-engine NOP
# carriers immediately before the instruction (identical stall semantics).
# ---------------------------------------------------------------------------
_MAX_WAITS = 1


def _split_waits(tc, inst):
    si = inst.sync_info
    if si is None or si.on_wait is None or len(si.on_wait) <= _MAX_WAITS:
        return
    waits = list(si.on_wait)
    keep = waits[-_MAX_WAITS:]
    hoist = waits[:-_MAX_WAITS]
    inst.sync_info = mybir.SyncInfo(on_wait=keep, on_update=list(si.on_update or []))
    eng = tc.nc.engines[inst.engine]
    for w in hoist:
        nop = eng.nop(nofuse=True)
        nop.ins.sync_info = mybir.SyncInfo(on_wait=[w], on_update=[])


_orig_commit = tile.TileContext._commit_instruction


def _commit_instruction_split(self, inst, lazy_reg_writes: bool = True):
    si = inst.sync_info
    if (
        si is not None
        and si.on_wait is not None
        and len(si.on_wait) > _MAX_WAITS
        and inst.engine != mybir.EngineType.Unassigned
    ):
        _split_waits(self, inst)
    return _orig_commit(self, inst, lazy_reg_writes)


tile.TileContext._commit_instruction = _commit_instruction_split


def _drain_and_barrier_split(self, tick_clock, wait_clock):
    drain_inst = self.nc.sync.drain()
    wait_clock.add_sem_waits(
        drain_inst.ins, ScopedClock({None: tick_clock.global_clock})
    )
    si = drain_inst.ins.sync_info
    if si is not None and si.on_wait is not None and len(si.on_wait) > _MAX_WAITS:
        waits = list(si.on_wait)
        drain_inst.ins.sync_info = mybir.SyncInfo(
            on_wait=[waits[0]], on_update=list(si.on_update or [])
        )
        for w in waits[1:]:
            extra = self.nc.sync.drain()
            extra.ins.sync_info = mybir.SyncInfo(on_wait=[w], on_update=[])

    self.nc.all_engine_barrier()
    assert self.sems is not None
    popped = self.nc._tile_sem_poison_stack.pop()
    assert popped is self._sem_poison
    self.nc.clear_and_free_semaphores(list(self.sems.allocated().values()))
    self.nc.all_engine_barrier()


tile.TileContext._drain_and_barrier = _drain_and_barrier_split

# ---------------------------------------------------------------------------
# Patch 2: provide antenv.axon_hooks (absent in this image) so that
# run_bass_kernel_spmd(trace=True) can NTFF-profile via libaxon_pjrt.so.
# ---------------------------------------------------------------------------
if "antenv.axon_hooks" not in sys.modules:
    _hookmod = types.ModuleType("antenv.axon_hooks")
    _hookmod._hook = None

    def _set_hook(h, _m=_hookmod):
        _m._hook = h

    def _get_hook(_m=_hookmod):
        return _m._hook

    _hookmod.set_axon_ntff_profile_hook = _set_hook
    _hookmod.get_axon_ntff_profile_hook = _get_hook
    sys.modules["antenv.axon_hooks"] = _hookmod
    try:
        from trn_agent_boot.trn_boot import _ntff_profile_via_ctypes

        _set_hook(_ntff_profile_via_ctypes("/opt/axon/libaxon_pjrt.so"))
    except Exception:
        pass


# ---------------------------------------------------------------------------
# Kernel build
# ---------------------------------------------------------------------------
def _build():
    nc = bass.Bass("TRN2", target_bir_lowering=False, debug=False,
                   num_devices=NCORES)

    q_d = nc.dram_tensor("q", [SH, D], F32, kind="ExternalInput")
    k_d = nc.dram_tensor("k", [S, D], F32, kind="ExternalInput")
    v_d = nc.dram_tensor("v", [S, D], F32, kind="ExternalInput")
    wq_d = nc.dram_tensor("wq", [D, D], F32R, kind="ExternalInput")
    wk_d = nc.dram_tensor("wk", [D, D], F32R, kind="ExternalInput")
    wv_d = nc.dram_tensor("wv", [D, D], F32R, kind="ExternalInput")
    bq_d = nc.dram_tensor("bq", [128, DC], F32, kind="ExternalInput")
    bk_d = nc.dram_tensor("bk", [128, DC], F32, kind="ExternalInput")
    bv_d = nc.dram_tensor("bv", [1, D], F32, kind="ExternalInput")
    id_d = nc.dram_tensor("ident", [128, 128], F32, kind="ExternalInput")
    scores_d = nc.dram_tensor("scores", [SH, S], F32, kind="ExternalOutput")
    out_d = nc.dram_tensor("out", [SH, D], F32, kind="ExternalOutput")

    with tile.TileContext(nc) as tc, ExitStack() as ctx:
        consts = ctx.enter_context(tc.tile_pool(name="consts", bufs=1))
        ident = consts.tile([128, 128], F32)
        nc.sync.dma_start(out=ident, in_=id_d[:, :])
        bq_sb = consts.tile([128, DC], F32, tag="bq")
        nc.sync.dma_start(out=bq_sb, in_=bq_d[:, :])
        bk_sb = consts.tile([128, DC], F32, tag="bk")
        nc.sync.dma_start(out=bk_sb, in_=bk_d[:, :])

        acts = ctx.enter_context(tc.tile_pool(name="acts", bufs=1))
        qhT = acts.tile([128, DC, SH], F32R, tag="qhT")
        khT = acts.tile([128, DC, S], F32R, tag="khT")
        vh = acts.tile([128, S // 128, D], F32R, tag="vh")

        # -------- Phase A: projections (q, k -> xhT; v -> vh natural) ------
        # Single rotating weight slot (bufs=2) so wk/wv DMA overlaps compute.
        with tc.tile_pool(name="wpool", bufs=2) as wpool, \
             tc.tile_pool(name="work1", bufs=2) as work, \
             tc.tile_pool(name="work2", bufs=2) as work2, \
             tc.tile_pool(name="psA", bufs=2, space="PSUM") as psA:

            for src, dst, w_d, bias_sb, nch in (
                (q_d, qhT, wq_d, bq_sb, SH // 256),
                (k_d, khT, wk_d, bk_sb, S // 256),
            ):
                w_sb = None
                for ch in range(nch):
                    nat = work.tile([128, 2, D], F32, tag="nat")
                    nc.sync.dma_start(
                        out=nat,
                        in_=src[ch * 256:(ch + 1) * 256, :]
                        .rearrange("(c p) e -> p c e", p=128),
                    )
                    if w_sb is None:
                        # emitted after the first data chunk so the chunk's
                        # DMA wins queue priority and PE starts sooner
                        w_sb = wpool.tile([128, DC, D], F32R, tag="w")
                        nc.sync.dma_start(
                            out=w_sb,
                            in_=w_d[:, :].rearrange("(dc p) e -> p dc e", p=128),
                        )
                    xT = work.tile([128, DC, 256], F32R, tag="xT")
                    for dc in range(DC):
                        pst = psA.tile([128, 256], F32, tag="pst")
                        for c in range(2):
                            nc.tensor.transpose(
                                pst[:, c * 128:(c + 1) * 128],
                                nat[:, c, dc * 128:(dc + 1) * 128],
                                ident,
                            )
                        nc.vector.tensor_copy(xT[:, dc, :], pst.bitcast(F32R))
                    for ec in range(DC):
                        psp = psA.tile([128, 256], F32, tag="psp")
                        for dc in range(DC):
                            nc.tensor.matmul(
                                psp,
                                w_sb[:, dc, ec * 128:(ec + 1) * 128],
                                xT[:, dc, :],
                                start=(dc == 0),
                                stop=(dc == DC - 1),
                            )
                        nc.vector.tensor_scalar_add(
                            dst[:, ec, ch * 256:(ch + 1) * 256],
                            psp.bitcast(F32R),
                            bias_sb[:, ec:ec + 1],
                        )

            # v projection -> vh natural [k, e]
            bv_rep = work2.tile([128, D], F32, tag="bv")
            nc.gpsimd.dma_start(
                out=bv_rep,
                in_=bass.AP(tensor=bv_d, offset=0, ap=[[0, 128], [1, D]]),
            )
            wv_sb = None
            for kc in range(S // 128):
                natv = work2.tile([128, D], F32, tag="natv")
                nc.sync.dma_start(out=natv, in_=v_d[kc * 128:(kc + 1) * 128, :])
                if wv_sb is None:
                    wv_sb = wpool.tile([128, DC, D], F32R, tag="w")
                    nc.sync.dma_start(
                        out=wv_sb,
                        in_=wv_d[:, :].rearrange("(dc p) e -> p dc e", p=128),
                    )
                vT = work2.tile([128, DC, 128], F32R, tag="vT")
                for g in range(2):
                    pst = psA.tile([128, 384], F32, tag="pst")
                    for j in range(3):
                        dc = g * 3 + j
                        nc.tensor.transpose(
                            pst[:, j * 128:(j + 1) * 128],
                            natv[:, dc * 128:(dc + 1) * 128],
                            ident,
                        )
                    nc.vector.tensor_copy(
                        vT[:, g * 3:(g + 1) * 3, :],
                        pst.rearrange("p (c x) -> p c x", c=3).bitcast(F32R),
                    )
                for ev in range(2):
                    psv = psA.tile([128, 384], F32, tag="psv")
                    for dc in range(DC):
                        nc.tensor.matmul(
                            psv,
                            vT[:, dc, :],
                            wv_sb[:, dc, ev * 384:(ev + 1) * 384],
                            start=(dc == 0),
                            stop=(dc == DC - 1),
                        )
                    nc.vector.tensor_add(
                        vh[:, kc, ev * 384:(ev + 1) * 384],
                        psv.bitcast(F32R),
                        bv_rep[:, ev * 384:(ev + 1) * 384].bitcast(F32R),
                    )

        # ---------------- Phase B: attention per q-tile --------------------
        with tc.tile_pool(name="att", bufs=2) as aw, \
             tc.tile_pool(name="psS", bufs=1, space="PSUM") as psS, \
             tc.tile_pool(name="psT2", bufs=2, space="PSUM") as psT2, \
             tc.tile_pool(name="psO", bufs=1, space="PSUM") as psO:
            NKG = S // 512
            for qt in range(SH // 128):
                pss = psS.tile([128, S], F32, tag="pss")
                scs = aw.tile([128, S], F32, tag="scs")
                probs = aw.tile([128, S], F32, tag="probs")
                den4 = aw.tile([128, NKG], F32, tag="den4")
                probsT = aw.tile([128, S // 128, 128], F32R, tag="probsT")
                for kg in range(NKG):
                    for ec in range(DC):
                        nc.tensor.matmul(
                            pss[:, kg * 512:(kg + 1) * 512],
                            qhT[:, ec, qt * 128:(qt + 1) * 128],
                            khT[:, ec, kg * 512:(kg + 1) * 512],
                            start=(ec == 0),
                            stop=(ec == DC - 1),
                        )
                    # per-512-block softmax front: copy + exp pipelined with
                    # the next block's matmuls
                    sl = slice(kg * 512, (kg + 1) * 512)
                    nc.scalar.copy(scs[:, sl], pss[:, sl])
                    nc.scalar.activation(probs[:, sl], scs[:, sl], AF.Exp,
                                         accum_out=den4[:, kg:kg + 1])
                    pst = psT2.tile([128, 512], F32, tag="pst2")
                    for j in range(4):
                        t = kg * 4 + j
                        nc.tensor.transpose(
                            pst[:, j * 128:(j + 1) * 128],
                            probs[:, t * 128:(t + 1) * 128],
                            ident,
                        )
                    nc.vector.tensor_copy(
                        probsT[:, kg * 4:(kg + 1) * 4, :],
                        pst.rearrange("p (c x) -> p c x", c=4).bitcast(F32R),
                    )
                nc.sync.dma_start(out=scores_d[qt * 128:(qt + 1) * 128, :], in_=scs)
                den = aw.tile([128, 1], F32, tag="den")
                nc.vector.reduce_sum(den, den4, axis=mybir.AxisListType.X)
                rden = aw.tile([128, 1], F32, tag="rden")
                nc.vector.reciprocal(rden, den)

                pso = psO.tile([128, D], F32, tag="pso")
                for e0, ew in ((0, 512), (512, 256)):
                    for t in range(S // 128):
                        nc.tensor.matmul(
                            pso[:, e0:e0 + ew],
                            probsT[:, t, :],
                            vh[:, t, e0:e0 + ew],
                            start=(t == 0),
                            stop=(t == S // 128 - 1),
                        )
                outsb = aw.tile([128, D], F32, tag="outsb")
                nc.vector.tensor_scalar_mul(outsb, pso, rden[:, 0:1])
                nc.sync.dma_start(out=out_d[qt * 128:(qt + 1) * 128, :], in_=outsb)

    return nc


_NC = None


def _get_nc():
    global _NC
    if _NC is None:
        _NC = _build()
    return _NC


def kernel(q, k, v, Wq, bq, Wk, bk, Wv, bv):
    q = np.ascontiguousarray(np.asarray(q, dtype=np.float32))
    k = np.ascontiguousarray(np.asarray(k, dtype=np.float32))
    v = np.ascontiguousarray(np.asarray(v, dtype=np.float32))
    Wq = np.asarray(Wq, dtype=np.float32)
    Wk = np.asarray(Wk, dtype=np.float32)
    Wv = np.asarray(Wv, dtype=np.float32)
    bq = np.asarray(bq, dtype=np.float32)
    bk = np.asarray(bk, dtype=np.float32)
    bv = np.asarray(bv, dtype=np.float32)

    wq_p = np.ascontiguousarray((Wq / SCALE).T)          # [d, e]
    wk_p = np.ascontiguousarray(Wk.T)
    wv_p = np.ascontiguousarray(Wv.T)
    bq_p = np.ascontiguousarray((bq / SCALE).reshape(DC, 128).T)   # [128, DC]
    bk_p = np.ascontiguousarray(bk.reshape(DC, 128).T)
    bv_p = np.ascontiguousarray(bv.reshape(1, D))
    ident = np.eye(128, dtype=np.float32)

    in_maps = []
    for c in range(NCORES):
        b, h = divmod(c, 2)
        in_maps.append({
            "q": np.ascontiguousarray(q[b, h * SH:(h + 1) * SH, :]),
            "k": k[b],
            "v": v[b],
            "wq": wq_p, "wk": wk_p, "wv": wv_p,
            "bq": bq_p, "bk": bk_p, "bv": bv_p,
            "ident": ident,
        })

    nc = _get_nc()
    trace = bool(int(os.environ.get("ATTN_TRACE", "0")))
    res = run_bass_kernel_spmd(nc, in_maps, list(range(NCORES)), trace=trace)
    if trace:
        print("HW exec time: %s ns" % res.exec_time_ns)
        kernel.last_results = res

    outputs = np.empty((B, S, D), dtype=np.float32)
    scores = np.empty((B, S, S), dtype=np.float32)
    for c in range(NCORES):
        b, h = divmod(c, 2)
        outputs[b, h * SH:(h + 1) * SH, :] = res.results[c]["out"]
        scores[b, h * SH:(h + 1) * SH, :] = res.results[c]["scores"]
    return (outputs, scores)


# revision 8
# speedup vs baseline: 1.2042x; 1.0760x over previous
"""Trainium2 Bass kernel for nn_Attention_47029891891630.

Single-head attention: qh = q@Wq.T+bq, kh = k@Wk.T+bk, vh = v@Wv.T+bv,
scores = qh@kh.T/sqrt(768), probs = softmax(scores), out = probs@vh.
Returns (out, scores) at full shape [4,2048,768] / [4,2048,2048].

Sharding: 8 cores = (batch b, q-half h); each core computes kh/vh for its
batch (redundantly within the pair) and attention for its 1024 q rows.

Per-core dataflow (all matmuls in float32r = full-rate reduced-precision):
  Phase A: DMA q/k/v natural -> PE-transpose 128x128 tiles -> project
           -> resident qhT [e,1024], khT [e,2048] (f32r), vh [k,e] (f32r).
           1/sqrt(D) is folded into Wq/bq on the host.
  Phase B: per 128-row q-tile: scores(psum) = qhT.khT -> ACT copy->HBM,
           ACT Exp(accum_out=rowsum) -> PE-transpose probs -> out(psum) =
           probsT.vh -> DVE mul by 1/rowsum -> HBM.
"""

import os
import sys
import types
from contextlib import ExitStack

import numpy as np

import concourse.bass as bass
import concourse.mybir as mybir
import concourse.tile as tile
from concourse.bass_utils import run_bass_kernel_spmd
from concourse.vector_clock import ScopedClock

F32 = mybir.dt.float32
F32R = mybir.dt.float32r
BF16 = mybir.dt.bfloat16
AF = mybir.ActivationFunctionType

B, S, D = 4, 2048, 768
SH = S // 2          # q rows per core
DC = D // 128        # 6 feature chunks
NCORES = 8
SCALE = float(np.sqrt(D))

# ---------------------------------------------------------------------------
# Patch 1: this container's walrus rejects >1 sync-wait per instruction
# ("Too many sync wait commands"). Hoist excess waits onto same-engine NOP
# carriers immediately before the instruction (identical stall semantics).
# ---------------------------------------------------------------------------
_MAX_WAITS = 1


def _split_waits(tc, inst):
    si = inst.sync_info
    if si is None or si.on_wait is None or len(si.on_wait) <= _MAX_WAITS:
        return
    waits = list(si.on_wait)
    keep = waits[-_MAX_WAITS:]
    hoist = waits[:-_MAX_WAITS]
    inst.sync_info = mybir.SyncInfo(on_wait=keep, on_update=list(si.on_update or []))
    eng = tc.nc.engines[inst.engine]
    for w in hoist:
        nop = eng.nop(nofuse=True)
        nop.ins.sync_info = mybir.SyncInfo(on_wait=[w], on_update=[])


_orig_commit = tile.TileContext._commit_instruction


def _commit_instruction_split(self, inst, lazy_reg_writes: bool = True):
    si = inst.sync_info
    if (
        si is not None
        and si.on_wait is not None
        and len(si.on_wait) > _MAX_WAITS
        and inst.engine != mybir.EngineType.Unassigned
    ):
        _split_waits(self, inst)
    return _orig_commit(self, inst, lazy_reg_writes)


tile.TileContext._commit_instruction = _commit_instruction_split


def _drain_and_barrier_split(self, tick_clock, wait_clock):
    drain_inst = self.nc.sync.drain()
    wait_clock.add_sem_waits(
        drain_inst.ins, ScopedClock({None: tick_clock.global_clock})
    )
    si = drain_inst.ins.sync_info
    if si is not None and si.on_wait is not None and len(si.on_wait) > _MAX_WAITS:
        waits = list(si.on_wait)
        drain_inst.ins.sync_info = mybir.SyncInfo(
            on_wait=[waits[0]], on_update=list(si.on_update or [])
        )
        for w in waits[1:]:
            extra = self.nc.sync.drain()
            extra.ins.sync_info = mybir.SyncInfo(on_wait=[w], on_update=[])

    self.nc.all_engine_barrier()
    assert self.sems is not None
    popped = self.nc._tile_sem_poison_stack.pop()
    assert popped is self._sem_poison
    self.nc.clear_and_free_semaphores(list(self.sems.allocated().values()))
    self.nc.all_engine_barrier()


tile.TileContext._drain_and_barrier = _drain_and_barrier_split

# ---------------------------------------------------------------------------
# Patch 2: provide antenv.axon_hooks (absent in this image) so that
# run_bass_kernel_spmd(trace=True) can NTFF-profile via libaxon_pjrt.so.
# ---------------------------------------------------------------------------
if "antenv.axon_hooks" not in sys.modules:
    _hookmod = types.ModuleType("antenv.axon_hooks")
    _hookmod._hook = None

    def _set_hook(h, _m=_hookmod):
        _m._hook = h

    def _get_hook(_m=_hookmod):
        return _m._hook

    _hookmod.set_axon_ntff_profile_hook = _set_hook
    _hookmod.get_axon_ntff_profile_hook = _get_hook
    sys.modules["antenv.axon_hooks"] = _hookmod
    try:
        from trn_agent_boot.trn_boot import _ntff_profile_via_ctypes

        _set_hook(_ntff_profile_via_ctypes("/opt/axon/libaxon_pjrt.so"))
    except Exception:
        pass


# ---------------------------------------------------------------------------
# Kernel build
# ---------------------------------------------------------------------------
def _build():
    nc = bass.Bass("TRN2", target_bir_lowering=False, debug=False,
                   num_devices=NCORES)

    q_d = nc.dram_tensor("q", [SH, D], BF16, kind="ExternalInput")
    k_d = nc.dram_tensor("k", [S, D], BF16, kind="ExternalInput")
    v_d = nc.dram_tensor("v", [S, D], BF16, kind="ExternalInput")
    wq_d = nc.dram_tensor("wq", [D, D], F32R, kind="ExternalInput")
    wk_d = nc.dram_tensor("wk", [D, D], F32R, kind="ExternalInput")
    wv_d = nc.dram_tensor("wv", [D, D], F32R, kind="ExternalInput")
    bq_d = nc.dram_tensor("bq", [128, DC], F32, kind="ExternalInput")
    bk_d = nc.dram_tensor("bk", [128, DC], F32, kind="ExternalInput")
    bv_d = nc.dram_tensor("bv", [1, D], F32, kind="ExternalInput")
    id_d = nc.dram_tensor("ident", [128, 128], BF16, kind="ExternalInput")
    scores_d = nc.dram_tensor("scores", [SH, S], F32, kind="ExternalOutput")
    out_d = nc.dram_tensor("out", [SH, D], F32, kind="ExternalOutput")

    with tile.TileContext(nc) as tc, ExitStack() as ctx:
        consts = ctx.enter_context(tc.tile_pool(name="consts", bufs=1))
        ident = consts.tile([128, 128], BF16)
        nc.sync.dma_start(out=ident, in_=id_d[:, :])
        bq_sb = consts.tile([128, DC], F32, tag="bq")
        nc.sync.dma_start(out=bq_sb, in_=bq_d[:, :])
        bk_sb = consts.tile([128, DC], F32, tag="bk")
        nc.sync.dma_start(out=bk_sb, in_=bk_d[:, :])

        acts = ctx.enter_context(tc.tile_pool(name="acts", bufs=1))
        qhT = acts.tile([128, DC, SH], F32R, tag="qhT")
        khT = acts.tile([128, DC, S], F32R, tag="khT")
        vh = acts.tile([128, S // 128, D], F32R, tag="vh")

        # -------- Phase A: projections (q, k -> xhT; v -> vh natural) ------
        # Single rotating weight slot (bufs=2) so wk/wv DMA overlaps compute.
        with tc.tile_pool(name="wpool", bufs=2) as wpool, \
             tc.tile_pool(name="work1", bufs=2) as work, \
             tc.tile_pool(name="work2", bufs=2) as work2, \
             tc.tile_pool(name="psA", bufs=2, space="PSUM") as psA:

            for src, dst, w_d, bias_sb, nch in (
                (q_d, qhT, wq_d, bq_sb, SH // 256),
                (k_d, khT, wk_d, bk_sb, S // 256),
            ):
                w_sb = None
                for ch in range(nch):
                    nat = work.tile([128, 2, D], BF16, tag="nat")
                    nc.sync.dma_start(
                        out=nat,
                        in_=src[ch * 256:(ch + 1) * 256, :]
                        .rearrange("(c p) e -> p c e", p=128),
                    )
                    if w_sb is None:
                        # emitted after the first data chunk so the chunk's
                        # DMA wins queue priority and PE starts sooner
                        w_sb = wpool.tile([128, DC, D], F32R, tag="w")
                        nc.sync.dma_start(
                            out=w_sb,
                            in_=w_d[:, :].rearrange("(dc p) e -> p dc e", p=128),
                        )
                    xT = work.tile([128, DC, 256], F32R, tag="xT")
                    for dc in range(DC):
                        pst = psA.tile([128, 256], BF16, tag="pst")
                        for c in range(2):
                            nc.tensor.transpose(
                                pst[:, c * 128:(c + 1) * 128],
                                nat[:, c, dc * 128:(dc + 1) * 128],
                                ident,
                            )
                        nc.vector.tensor_copy(xT[:, dc, :], pst)
                    for ec in range(DC):
                        psp = psA.tile([128, 256], F32, tag="psp")
                        for dc in range(DC):
                            nc.tensor.matmul(
                                psp,
                                w_sb[:, dc, ec * 128:(ec + 1) * 128],
                                xT[:, dc, :],
                                start=(dc == 0),
                                stop=(dc == DC - 1),
                            )
                        nc.vector.tensor_scalar_add(
                            dst[:, ec, ch * 256:(ch + 1) * 256],
                            psp.bitcast(F32R),
                            bias_sb[:, ec:ec + 1],
                        )

            # v projection -> vh natural [k, e]
            bv_rep = work2.tile([128, D], F32, tag="bv")
            nc.gpsimd.dma_start(
                out=bv_rep,
                in_=bass.AP(tensor=bv_d, offset=0, ap=[[0, 128], [1, D]]),
            )
            wv_sb = None
            for kc in range(S // 128):
                natv = work2.tile([128, D], BF16, tag="natv")
                nc.sync.dma_start(out=natv, in_=v_d[kc * 128:(kc + 1) * 128, :])
                if wv_sb is None:
                    wv_sb = wpool.tile([128, DC, D], F32R, tag="w")
                    nc.sync.dma_start(
                        out=wv_sb,
                        in_=wv_d[:, :].rearrange("(dc p) e -> p dc e", p=128),
                    )
                vT = work2.tile([128, DC, 128], F32R, tag="vT")
                for g in range(2):
                    pst = psA.tile([128, 384], BF16, tag="pst")
                    for j in range(3):
                        dc = g * 3 + j
                        nc.tensor.transpose(
                            pst[:, j * 128:(j + 1) * 128],
                            natv[:, dc * 128:(dc + 1) * 128],
                            ident,
                        )
                    nc.vector.tensor_copy(
                        vT[:, g * 3:(g + 1) * 3, :],
                        pst.rearrange("p (c x) -> p c x", c=3),
                    )
                for ev in range(2):
                    psv = psA.tile([128, 384], F32, tag="psv")
                    for dc in range(DC):
                        nc.tensor.matmul(
                            psv,
                            vT[:, dc, :],
                            wv_sb[:, dc, ev * 384:(ev + 1) * 384],
                            start=(dc == 0),
                            stop=(dc == DC - 1),
                        )
                    nc.vector.tensor_add(
                        vh[:, kc, ev * 384:(ev + 1) * 384],
                        psv.bitcast(F32R),
                        bv_rep[:, ev * 384:(ev + 1) * 384].bitcast(F32R),
                    )

        # ---------------- Phase B: attention per q-tile --------------------
        with tc.tile_pool(name="att", bufs=2) as aw, \
             tc.tile_pool(name="psS", bufs=1, space="PSUM") as psS, \
             tc.tile_pool(name="psT2", bufs=2, space="PSUM") as psT2, \
             tc.tile_pool(name="psO", bufs=1, space="PSUM") as psO:
            NKG = S // 512
            for qt in range(SH // 128):
                pss = psS.tile([128, S], F32, tag="pss")
                scs = aw.tile([128, S], F32, tag="scs")
                probs = aw.tile([128, S], BF16, tag="probs")
                den4 = aw.tile([128, NKG], F32, tag="den4")
                probsT = aw.tile([128, S // 128, 128], F32R, tag="probsT")
                for kg in range(NKG):
                    for ec in range(DC):
                        nc.tensor.matmul(
                            pss[:, kg * 512:(kg + 1) * 512],
                            qhT[:, ec, qt * 128:(qt + 1) * 128],
                            khT[:, ec, kg * 512:(kg + 1) * 512],
                            start=(ec == 0),
                            stop=(ec == DC - 1),
                        )
                    # per-512-block softmax front: copy + exp pipelined with
                    # the next block's matmuls
                    sl = slice(kg * 512, (kg + 1) * 512)
                    nc.scalar.copy(scs[:, sl], pss[:, sl])
                    nc.scalar.activation(probs[:, sl], scs[:, sl], AF.Exp,
                                         accum_out=den4[:, kg:kg + 1])
                    pst = psT2.tile([128, 512], BF16, tag="pst2")
                    for j in range(4):
                        t = kg * 4 + j
                        nc.tensor.transpose(
                            pst[:, j * 128:(j + 1) * 128],
                            probs[:, t * 128:(t + 1) * 128],
                            ident,
                        )
                    nc.vector.tensor_copy(
                        probsT[:, kg * 4:(kg + 1) * 4, :],
                        pst.rearrange("p (c x) -> p c x", c=4),
                    )
                nc.sync.dma_start(out=scores_d[qt * 128:(qt + 1) * 128, :], in_=scs)
                den = aw.tile([128, 1], F32, tag="den")
                nc.vector.reduce_sum(den, den4, axis=mybir.AxisListType.X)
                rden = aw.tile([128, 1], F32, tag="rden")
                nc.vector.reciprocal(rden, den)

                pso = psO.tile([128, D], F32, tag="pso")
                for e0, ew in ((0, 512), (512, 256)):
                    for t in range(S // 128):
                        nc.tensor.matmul(
                            pso[:, e0:e0 + ew],
                            probsT[:, t, :],
                            vh[:, t, e0:e0 + ew],
                            start=(t == 0),
                            stop=(t == S // 128 - 1),
                        )
                outsb = aw.tile([128, D], F32, tag="outsb")
                nc.vector.tensor_scalar_mul(outsb, pso, rden[:, 0:1])
                nc.sync.dma_start(out=out_d[qt * 128:(qt + 1) * 128, :], in_=outsb)

    return nc


_NC = None


def _get_nc():
    global _NC
    if _NC is None:
        _NC = _build()
    return _NC


def kernel(q, k, v, Wq, bq, Wk, bk, Wv, bv):
    import ml_dtypes
    bf16 = ml_dtypes.bfloat16
    q = np.ascontiguousarray(np.asarray(q, dtype=np.float32).astype(bf16))
    k = np.ascontiguousarray(np.asarray(k, dtype=np.float32).astype(bf16))
    v = np.ascontiguousarray(np.asarray(v, dtype=np.float32).astype(bf16))
    Wq = np.asarray(Wq, dtype=np.float32)
    Wk = np.asarray(Wk, dtype=np.float32)
    Wv = np.asarray(Wv, dtype=np.float32)
    bq = np.asarray(bq, dtype=np.float32)
    bk = np.asarray(bk, dtype=np.float32)
    bv = np.asarray(bv, dtype=np.float32)

    wq_p = np.ascontiguousarray((Wq / SCALE).T)          # [d, e]
    wk_p = np.ascontiguousarray(Wk.T)
    wv_p = np.ascontiguousarray(Wv.T)
    bq_p = np.ascontiguousarray((bq / SCALE).reshape(DC, 128).T)   # [128, DC]
    bk_p = np.ascontiguousarray(bk.reshape(DC, 128).T)
    bv_p = np.ascontiguousarray(bv.reshape(1, D))
    ident = np.eye(128, dtype=np.float32).astype(bf16)

    in_maps = []
    for c in range(NCORES):
        b, h = divmod(c, 2)
        in_maps.append({
            "q": np.ascontiguousarray(q[b, h * SH:(h + 1) * SH, :]),
            "k": k[b],
            "v": v[b],
            "wq": wq_p, "wk": wk_p, "wv": wv_p,
            "bq": bq_p, "bk": bk_p, "bv": bv_p,
            "ident": ident,
        })

    nc = _get_nc()
    trace = bool(int(os.environ.get("ATTN_TRACE", "0")))
    res = run_bass_kernel_spmd(nc, in_maps, list(range(NCORES)), trace=trace)
    if trace:
        print("HW exec time: %s ns" % res.exec_time_ns)
        kernel.last_results = res

    outputs = np.empty((B, S, D), dtype=np.float32)
    scores = np.empty((B, S, S), dtype=np.float32)
    for c in range(NCORES):
        b, h = divmod(c, 2)
        outputs[b, h * SH:(h + 1) * SH, :] = res.results[c]["out"]
        scores[b, h * SH:(h + 1) * SH, :] = res.results[c]["scores"]
    return (outputs, scores)


# revision 23
# speedup vs baseline: 1.3441x; 1.1162x over previous
"""Trainium2 Bass kernel for nn_Attention_47029891891630.

Single-head attention: qh = q@Wq.T+bq, kh = k@Wk.T+bk, vh = v@Wv.T+bv,
scores = qh@kh.T/sqrt(768), probs = softmax(scores), out = probs@vh.
Returns (out, scores) at full shape [4,2048,768] / [4,2048,2048].

Sharding: 8 cores = (batch b, q-half h); each core computes kh/vh for its
batch (redundantly within the pair) and attention for its 1024 q rows.

Per-core dataflow (all matmuls in float32r = full-rate reduced-precision):
  Phase A: DMA q/k/v natural -> PE-transpose 128x128 tiles -> project
           -> resident qhT [e,1024], khT [e,2048] (f32r), vh [k,e] (f32r).
           1/sqrt(D) is folded into Wq/bq on the host.
  Phase B: per 128-row q-tile: scores(psum) = qhT.khT -> ACT copy->HBM,
           ACT Exp(accum_out=rowsum) -> PE-transpose probs -> out(psum) =
           probsT.vh -> DVE mul by 1/rowsum -> HBM.
"""

import os
import sys
import types
from contextlib import ExitStack

import numpy as np

import concourse.bass as bass
import concourse.mybir as mybir
import concourse.tile as tile
from concourse.bass_utils import run_bass_kernel_spmd
from concourse.vector_clock import ScopedClock

F32 = mybir.dt.float32
F32R = mybir.dt.float32r
BF16 = mybir.dt.bfloat16
AF = mybir.ActivationFunctionType

B, S, D = 4, 2048, 768
SH = S // 2          # q rows per core
DC = D // 128        # 6 feature chunks
NCORES = 8
SCALE = float(np.sqrt(D))

# ---------------------------------------------------------------------------
# Patch 1: this container's walrus rejects >1 sync-wait per instruction
# ("Too many sync wait commands"). Hoist excess waits onto same-engine NOP
# carriers immediately before the instruction (identical stall semantics).
# ---------------------------------------------------------------------------
_MAX_WAITS = 1


def _split_waits(tc, inst):
    si = inst.sync_info
    if si is None or si.on_wait is None or len(si.on_wait) <= _MAX_WAITS:
        return
    waits = list(si.on_wait)
    keep = waits[-_MAX_WAITS:]
    hoist = waits[:-_MAX_WAITS]
    inst.sync_info = mybir.SyncInfo(on_wait=keep, on_update=list(si.on_update or []))
    eng = tc.nc.engines[inst.engine]
    for w in hoist:
        nop = eng.nop(nofuse=True)
        nop.ins.sync_info = mybir.SyncInfo(on_wait=[w], on_update=[])


_orig_commit = tile.TileContext._commit_instruction


def _commit_instruction_split(self, inst, lazy_reg_writes: bool = True):
    si = inst.sync_info
    if (
        si is not None
        and si.on_wait is not None
        and len(si.on_wait) > _MAX_WAITS
        and inst.engine != mybir.EngineType.Unassigned
    ):
        _split_waits(self, inst)
    return _orig_commit(self, inst, lazy_reg_writes)


tile.TileContext._commit_instruction = _commit_instruction_split


def _drain_and_barrier_split(self, tick_clock, wait_clock):
    drain_inst = self.nc.sync.drain()
    wait_clock.add_sem_waits(
        drain_inst.ins, ScopedClock({None: tick_clock.global_clock})
    )
    si = drain_inst.ins.sync_info
    if si is not None and si.on_wait is not None and len(si.on_wait) > _MAX_WAITS:
        waits = list(si.on_wait)
        drain_inst.ins.sync_info = mybir.SyncInfo(
            on_wait=[waits[0]], on_update=list(si.on_update or [])
        )
        for w in waits[1:]:
            extra = self.nc.sync.drain()
            extra.ins.sync_info = mybir.SyncInfo(on_wait=[w], on_update=[])

    self.nc.all_engine_barrier()
    assert self.sems is not None
    popped = self.nc._tile_sem_poison_stack.pop()
    assert popped is self._sem_poison
    self.nc.clear_and_free_semaphores(list(self.sems.allocated().values()))
    self.nc.all_engine_barrier()


tile.TileContext._drain_and_barrier = _drain_and_barrier_split

# ---------------------------------------------------------------------------
# Patch 2: provide antenv.axon_hooks (absent in this image) so that
# run_bass_kernel_spmd(trace=True) can NTFF-profile via libaxon_pjrt.so.
# ---------------------------------------------------------------------------
if "antenv.axon_hooks" not in sys.modules:
    _hookmod = types.ModuleType("antenv.axon_hooks")
    _hookmod._hook = None

    def _set_hook(h, _m=_hookmod):
        _m._hook = h

    def _get_hook(_m=_hookmod):
        return _m._hook

    _hookmod.set_axon_ntff_profile_hook = _set_hook
    _hookmod.get_axon_ntff_profile_hook = _get_hook
    sys.modules["antenv.axon_hooks"] = _hookmod
    try:
        from trn_agent_boot.trn_boot import _ntff_profile_via_ctypes

        _set_hook(_ntff_profile_via_ctypes("/opt/axon/libaxon_pjrt.so"))
    except Exception:
        pass


# ---------------------------------------------------------------------------
# Kernel build
# ---------------------------------------------------------------------------
def _build():
    nc = bass.Bass("TRN2", target_bir_lowering=False, debug=False,
                   num_devices=NCORES)

    q_d = nc.dram_tensor("q", [SH, D], F32, kind="ExternalInput")
    k_d = nc.dram_tensor("k", [SH, D], F32, kind="ExternalInput")
    v_d = nc.dram_tensor("v", [SH, D], F32, kind="ExternalInput")
    wq_d = nc.dram_tensor("wq", [D, D], F32R, kind="ExternalInput")
    wk_d = nc.dram_tensor("wk", [D, D], F32R, kind="ExternalInput")
    wv_d = nc.dram_tensor("wv", [D, D], F32R, kind="ExternalInput")
    bq_d = nc.dram_tensor("bq", [128, DC], F32, kind="ExternalInput")
    bk_d = nc.dram_tensor("bk", [128, DC], F32, kind="ExternalInput")
    bv_d = nc.dram_tensor("bv", [1, D], F32, kind="ExternalInput")
    id_d = nc.dram_tensor("ident", [128, 128], F32, kind="ExternalInput")
    scores_d = nc.dram_tensor("scores", [SH, S], F32, kind="ExternalOutput")
    out_d = nc.dram_tensor("out", [SH, D], F32, kind="ExternalOutput")

    with tile.TileContext(nc) as tc, ExitStack() as ctx:
        consts = ctx.enter_context(tc.tile_pool(name="consts", bufs=1))
        ident = consts.tile([128, 128], F32)
        nc.sync.dma_start(out=ident, in_=id_d[:, :])
        bq_sb = consts.tile([128, DC], F32, tag="bq")
        nc.sync.dma_start(out=bq_sb, in_=bq_d[:, :])
        bk_sb = consts.tile([128, DC], F32, tag="bk")
        nc.sync.dma_start(out=bk_sb, in_=bk_d[:, :])
        ident_bf = consts.tile([128, 128], BF16, tag="identbf")
        nc.vector.tensor_copy(ident_bf, ident)

        # qhT/khT allocated early (khT AllGather reads overlap q-projection);
        # vh allocated after phase-A working pools close (reads gate only the
        # first out-matmul, well into phase B).
        actsQK = ctx.enter_context(tc.tile_pool(name="actsQK", bufs=1))
        qhT = actsQK.tile([128, DC, SH], F32R, tag="qhT")
        khT = actsQK.tile([128, DC, S], F32R, tag="khT")

        # -------- Phase A: each core projects q (own 1024 rows) and its
        # HALF of k and v; pairs exchange halves via intra-pair AllGather.
        HKC = SH // 128   # 8 v-chunks of 128 rows (half)
        KELE = DC * SH    # 6144 f32r elems/partition for the khT half
        dramp = ctx.enter_context(tc.tile_pool(name="dram", bufs=1, space="DRAM"))
        cc_in = dramp.tile([128, KELE + HKC * D], F32R, tag="cc_in")
        cc_out = dramp.tile([2, 128, KELE + HKC * D], F32R, tag="cc_out")

        with tc.tile_pool(name="wpool", bufs=2) as wpool, \
             tc.tile_pool(name="work1", bufs=2) as work, \
             tc.tile_pool(name="work2", bufs=2) as work2, \
             tc.tile_pool(name="psA", bufs=3, space="PSUM") as psA:

            # ---- k half -> cc_in[:, 0:KELE] (layout [dc, s_local]) ----
            wk_sb = None
            for ch in range(SH // 256):
                nat = work.tile([128, 2, D], F32, tag="nat")
                nc.sync.dma_start(
                    out=nat,
                    in_=k_d[ch * 256:(ch + 1) * 256, :]
                    .rearrange("(c p) e -> p c e", p=128),
                )
                if wk_sb is None:
                    wk_sb = wpool.tile([128, DC, D], F32R, tag="w")
                    nc.sync.dma_start(
                        out=wk_sb,
                        in_=wk_d[:, :].rearrange("(dc p) e -> p dc e", p=128),
                    )
                xT = work.tile([128, DC, 256], F32R, tag="xT", bufs=3)
                for dc in range(DC):
                    pst = psA.tile([128, 256], F32, tag="pst")
                    for c in range(2):
                        nc.tensor.transpose(
                            pst[:, c * 128:(c + 1) * 128],
                            nat[:, c, dc * 128:(dc + 1) * 128],
                            ident,
                        )
                    nc.vector.tensor_copy(xT[:, dc, :], pst)
                kout = work.tile([128, DC, 256], F32R, tag="kout")
                for ec in range(DC):
                    psp = psA.tile([128, 256], F32, tag="psp")
                    for dc in range(DC):
                        nc.tensor.matmul(
                            psp,
                            wk_sb[:, dc, ec * 128:(ec + 1) * 128],
                            xT[:, dc, :],
                            start=(dc == 0),
                            stop=(dc == DC - 1),
                        )
                    nc.vector.tensor_scalar_add(
                        kout[:, ec, :], psp.bitcast(F32R), bk_sb[:, ec:ec + 1],
                    )
                nc.sync.dma_start(
                    out=cc_in[:, 0:KELE]
                    .rearrange("p (dc s) -> p dc s", dc=DC)[:, :, ch * 256:(ch + 1) * 256],
                    in_=kout,
                )

            # ---- v half -> cc_in[:, KELE:] (layout [k_local, e]) ----
            bv_rep = work2.tile([128, D], F32, tag="bv")
            nc.gpsimd.dma_start(
                out=bv_rep,
                in_=bass.AP(tensor=bv_d, offset=0, ap=[[0, 128], [1, D]]),
            )
            wv_sb = None
            for kc in range(HKC):
                natv = work2.tile([128, D], F32, tag="natv")
                nc.sync.dma_start(out=natv, in_=v_d[kc * 128:(kc + 1) * 128, :])
                if wv_sb is None:
                    wv_sb = wpool.tile([128, DC, D], F32R, tag="w")
                    wvv = wv_d[:, :].rearrange("(dc p) e -> p dc e", p=128)
                    nc.sync.dma_start(out=wv_sb[:, :, 0:D // 2],
                                      in_=wvv[:, :, 0:D // 2])
                    nc.sync.dma_start(out=wv_sb[:, :, D // 2:D],
                                      in_=wvv[:, :, D // 2:D])
                vT = work2.tile([128, DC, 128], F32R, tag="vT")
                for g in range(2):
                    pst = psA.tile([128, 384], F32, tag="pst")
                    for j in range(3):
                        dc = g * 3 + j
                        nc.tensor.transpose(
                            pst[:, j * 128:(j + 1) * 128],
                            natv[:, dc * 128:(dc + 1) * 128],
                            ident,
                        )
                    nc.vector.tensor_copy(
                        vT[:, g * 3:(g + 1) * 3, :],
                        pst.rearrange("p (c x) -> p c x", c=3),
                    )
                vout = work2.tile([128, D], F32R, tag="vout")
                for ev in range(2):
                    psv = psA.tile([128, 384], F32, tag="psp")
                    for dc in range(DC):
                        nc.tensor.matmul(
                            psv,
                            vT[:, dc, :],
                            wv_sb[:, dc, ev * 384:(ev + 1) * 384],
                            start=(dc == 0),
                            stop=(dc == DC - 1),
                        )
                    nc.vector.tensor_add(
                        vout[:, ev * 384:(ev + 1) * 384],
                        psv.bitcast(F32R),
                        bv_rep[:, ev * 384:(ev + 1) * 384].bitcast(F32R),
                    )
                nc.sync.dma_start(
                    out=cc_in[:, KELE + kc * D:KELE + (kc + 1) * D], in_=vout,
                )

            # ---- exchange halves within the core pair ----
            nc.gpsimd.collective_compute(
                "AllGather",
                mybir.AluOpType.bypass,
                replica_groups=[[0, 1], [2, 3], [4, 5], [6, 7]],
                ins=[cc_in.opt()],
                outs=[cc_out.opt()],
            )
            # khT reads in kg(512) order so phase B's kg-loop can pipeline
            for r in range(2):
                for j in range(2):
                    nc.sync.dma_start(
                        out=khT[:, :, r * SH + j * 512:r * SH + (j + 1) * 512],
                        in_=cc_out[r, :, 0:KELE]
                        .rearrange("p (dc s) -> p dc s", dc=DC)[:, :, j * 512:(j + 1) * 512],
                    )

            # ---- q projection -> qhT (overlaps the collective) ----
            wq_sb = None
            for ch in range(SH // 256):
                nat = work.tile([128, 2, D], F32, tag="nat")
                nc.sync.dma_start(
                    out=nat,
                    in_=q_d[ch * 256:(ch + 1) * 256, :]
                    .rearrange("(c p) e -> p c e", p=128),
                )
                if wq_sb is None:
                    wq_sb = wpool.tile([128, DC, D], F32R, tag="w")
                    nc.sync.dma_start(
                        out=wq_sb,
                        in_=wq_d[:, :].rearrange("(dc p) e -> p dc e", p=128),
                    )
                xT = work.tile([128, DC, 256], F32R, tag="xT", bufs=3)
                for dc in range(DC):
                    pst = psA.tile([128, 256], F32, tag="pst")
                    for c in range(2):
                        nc.tensor.transpose(
                            pst[:, c * 128:(c + 1) * 128],
                            nat[:, c, dc * 128:(dc + 1) * 128],
                            ident,
                        )
                    nc.vector.tensor_copy(xT[:, dc, :], pst)
                for ec in range(DC):
                    psp = psA.tile([128, 256], F32, tag="psp")
                    for dc in range(DC):
                        nc.tensor.matmul(
                            psp,
                            wq_sb[:, dc, ec * 128:(ec + 1) * 128],
                            xT[:, dc, :],
                            start=(dc == 0),
                            stop=(dc == DC - 1),
                        )
                    nc.vector.tensor_scalar_add(
                        qhT[:, ec, ch * 256:(ch + 1) * 256],
                        psp.bitcast(F32R),
                        bq_sb[:, ec:ec + 1],
                    )

        # vh pool opens after phase-A working pools close
        actsV = ctx.enter_context(tc.tile_pool(name="actsV", bufs=1))
        vh = actsV.tile([128, S // 128, D], F32R, tag="vh")
        for r in range(2):
            nc.sync.dma_start(
                out=vh[:, r * HKC:(r + 1) * HKC, :],
                in_=cc_out[r, :, KELE:]
                .rearrange("p (t e) -> p t e", t=HKC),
            )

        # ---------------- Phase B: attention per q-tile --------------------
        with tc.tile_pool(name="att", bufs=2) as aw, \
             tc.tile_pool(name="psS", bufs=1, space="PSUM") as psS, \
             tc.tile_pool(name="psT2", bufs=2, space="PSUM") as psT2, \
             tc.tile_pool(name="psO", bufs=1, space="PSUM") as psO:
            NKG = S // 512
            for qt in range(SH // 128):
                scs = aw.tile([128, S], F32, tag="scs")
                probs = aw.tile([128, S], BF16, tag="probs")
                den4 = aw.tile([128, NKG], F32, tag="den4")
                probsT = aw.tile([128, S // 128, 128], F32R, tag="probsT")
                for kg in range(NKG):
                    pss = psS.tile([128, 512], F32, tag="pss", bufs=3)
                    for ec in range(DC):
                        nc.tensor.matmul(
                            pss,
                            qhT[:, ec, qt * 128:(qt + 1) * 128],
                            khT[:, ec, kg * 512:(kg + 1) * 512],
                            start=(ec == 0),
                            stop=(ec == DC - 1),
                        )
                    # per-512-block softmax front: copy + exp pipelined with
                    # the next block's matmuls
                    sl = slice(kg * 512, (kg + 1) * 512)
                    nc.scalar.copy(scs[:, sl], pss)
                    nc.scalar.activation(probs[:, sl], scs[:, sl], AF.Exp,
                                         accum_out=den4[:, kg:kg + 1])
                    pst = psT2.tile([128, 512], BF16, tag="pst2")
                    for j in range(4):
                        t = kg * 4 + j
                        nc.tensor.transpose(
                            pst[:, j * 128:(j + 1) * 128],
                            probs[:, t * 128:(t + 1) * 128],
                            ident_bf,
                        )
                    nc.vector.tensor_copy(
                        probsT[:, kg * 4:(kg + 1) * 4, :],
                        pst.rearrange("p (c x) -> p c x", c=4),
                    )
                nc.sync.dma_start(out=scores_d[qt * 128:(qt + 1) * 128, :], in_=scs)
                den = aw.tile([128, 1], F32, tag="den")
                nc.vector.reduce_sum(den, den4, axis=mybir.AxisListType.X)
                rden = aw.tile([128, 1], F32, tag="rden")
                nc.vector.reciprocal(rden, den)

                outsb = aw.tile([128, D], F32, tag="outsb")
                for e0, ew in ((0, 512), (512, 256)):
                    pso = psO.tile([128, ew], F32, tag="pso", bufs=3)
                    for t in range(S // 128):
                        nc.tensor.matmul(
                            pso,
                            probsT[:, t, :],
                            vh[:, t, e0:e0 + ew],
                            start=(t == 0),
                            stop=(t == S // 128 - 1),
                        )
                    nc.vector.tensor_scalar_mul(outsb[:, e0:e0 + ew], pso,
                                                rden[:, 0:1])
                nc.sync.dma_start(out=out_d[qt * 128:(qt + 1) * 128, :], in_=outsb)

    return nc


_NC = None


def _get_nc():
    global _NC
    if _NC is None:
        _NC = _build()
    return _NC


def kernel(q, k, v, Wq, bq, Wk, bk, Wv, bv):
    q = np.ascontiguousarray(np.asarray(q, dtype=np.float32))
    k = np.ascontiguousarray(np.asarray(k, dtype=np.float32))
    v = np.ascontiguousarray(np.asarray(v, dtype=np.float32))
    Wq = np.asarray(Wq, dtype=np.float32)
    Wk = np.asarray(Wk, dtype=np.float32)
    Wv = np.asarray(Wv, dtype=np.float32)
    bq = np.asarray(bq, dtype=np.float32)
    bk = np.asarray(bk, dtype=np.float32)
    bv = np.asarray(bv, dtype=np.float32)

    wq_p = np.ascontiguousarray((Wq / SCALE).T)          # [d, e]
    wk_p = np.ascontiguousarray(Wk.T)
    wv_p = np.ascontiguousarray(Wv.T)
    bq_p = np.ascontiguousarray((bq / SCALE).reshape(DC, 128).T)   # [128, DC]
    bk_p = np.ascontiguousarray(bk.reshape(DC, 128).T)
    bv_p = np.ascontiguousarray(bv.reshape(1, D))
    ident = np.eye(128, dtype=np.float32)

    in_maps = []
    for c in range(NCORES):
        b, h = divmod(c, 2)
        in_maps.append({
            "q": np.ascontiguousarray(q[b, h * SH:(h + 1) * SH, :]),
            "k": np.ascontiguousarray(k[b, h * SH:(h + 1) * SH, :]),
            "v": np.ascontiguousarray(v[b, h * SH:(h + 1) * SH, :]),
            "wq": wq_p, "wk": wk_p, "wv": wv_p,
            "bq": bq_p, "bk": bk_p, "bv": bv_p,
            "ident": ident,
        })

    nc = _get_nc()
    trace = bool(int(os.environ.get("ATTN_TRACE", "0")))
    res = run_bass_kernel_spmd(nc, in_maps, list(range(NCORES)), trace=trace)
    if trace:
        print("HW exec time: %s ns" % res.exec_time_ns)
        kernel.last_results = res

    outputs = np.empty((B, S, D), dtype=np.float32)
    scores = np.empty((B, S, S), dtype=np.float32)
    for c in range(NCORES):
        b, h = divmod(c, 2)
        outputs[b, h * SH:(h + 1) * SH, :] = res.results[c]["out"]
        scores[b, h * SH:(h + 1) * SH, :] = res.results[c]["scores"]
    return (outputs, scores)
